# revision 51
# baseline (speedup 1.0000x reference)
"""Trainium2 Bass kernel for causal multi-head attention with interleaved RoPE.

Problem: B=2, S=2048, D=1024, 16 heads x 64 dims, causal, rope theta=1e4.

Sharding (8 cores): 2-way batch x 4-way head tensor-parallel.
  core i: batch b = i // 4, head group g = i % 4 (heads 4g..4g+3, dims 256).
  Each core computes q/k/v for its heads from x[b], runs causal flash
  attention, and produces a partial output projection outT = wo_g.T-slice
  contribution [D, S] in bf16.  Host sums the 4 partials per batch (f32) and
  transposes.

Performance structure (single gap-free PE stream to defeat the PE p-state
clock ramp):
  - pre-phase: K proj (g=0), V proj, Q proj (g=0), rope on DVE.
  - attention g=0, with the g=1 K/Q projection matmuls interleaved as PE
    fillers between attention rounds (they have no Scalar dependency, so
    they fill the EXP-bound gaps).
  - attention g=1, with the output-projection matmuls of completed q-tiles
    interleaved the same way.
  - Scores are computed transposed (S^T[k, q]) in chunk PAIRS sharing one
    2-bank PSUM tile so each Scalar EXP covers [128, 1024] (amortizes the
    ~185ns activation access bubble).  AV matmuls lag one pair behind the
    score matmuls (software pipeline), accumulating into a per-(g,qt,a)
    PSUM tile whose 65th row (ones column of v_aug) is the softmax
    normalizer.
  - Normalization: reciprocal_approx_fast of the sum rows, PE-matmul
    broadcast via a constant selector matrix, one DVE multiply per head
    half writing bf16 oT.
  - x / wq / wk / wv / wo travel as bf16 (halves DMA); q/k/scores/probs
    stay f32r.
"""

import os
import sys
from collections import deque

sys.path.insert(0, "/opt/trn_rl_repo")

import numpy as np

B = 2
S = 2048
D = 1024
NH = 16
HD = 64
THETA = 10000.0
NCORES = 8
HPC = 4  # heads per core
DC = HPC * HD  # 256 dims per core
GQ = 2  # 128-partition groups per core for q/k/o dims (DC/128)
QT = 512  # query tile (free dim)
NQT = S // QT
KC = 128  # key chunk (partition dim)
NOC = D // 128  # contraction chunks
MASKVAL = -60.0

_CACHE = {}


def _install_axon_ntff_hook():
    """Register antenv.axon_hooks so trace=True (BASS_TRACE=1) works."""
    import types

    if "antenv.axon_hooks" in sys.modules:
        return
    m = types.ModuleType("antenv.axon_hooks")
    _hook = [None]
    m.set_axon_ntff_profile_hook = lambda h: _hook.__setitem__(0, h)
    m.get_axon_ntff_profile_hook = lambda: _hook[0]
    sys.modules["antenv.axon_hooks"] = m
    try:
        import antenv

        antenv.axon_hooks = m
        from trn_agent_boot.trn_boot import _ntff_profile_via_ctypes

        hook = _ntff_profile_via_ctypes("/opt/axon/libaxon_pjrt.so")
        if hook is not None:
            m.set_axon_ntff_profile_hook(hook)
    except Exception:
        pass


def _rope_perm_local():
    """Permutation of one head's 64 dims: original interleaved pair (2i, 2i+1)
    -> t0 at quadrant*32 + (i%16), t1 at quadrant*32 + 16 + (i%16), with
    quadrant = i // 16.  Returns perm such that new[j] = old[perm[j]]."""
    perm = np.zeros(HD, dtype=np.int64)
    for i in range(HD // 2):
        qd, r = divmod(i, 16)
        perm[qd * 32 + r] = 2 * i
        perm[qd * 32 + 16 + r] = 2 * i + 1
    return perm


def _rope_tables():
    """cos_dup/sin_signed [128, S]: per-partition rope tables matching the
    de-interleaved layout (pattern repeats every 64 partitions)."""
    inv_freq = 1.0 / (THETA ** (np.arange(0, HD, 2, dtype=np.float64) / HD))  # [32]
    pos = np.arange(S, dtype=np.float64)
    ang = pos[None, :] * inv_freq[:, None]  # [32, S]
    cos = np.cos(ang)
    sin = np.sin(ang)
    cos_dup = np.zeros((128, S), dtype=np.float32)
    sin_signed = np.zeros((128, S), dtype=np.float32)
    for p in range(128):
        d = p % HD
        qd, r0 = divmod(d, 32)
        if r0 < 16:
            i = qd * 16 + r0
            cos_dup[p] = cos[i]
            sin_signed[p] = -sin[i]
        else:
            i = qd * 16 + (r0 - 16)
            cos_dup[p] = cos[i]
            sin_signed[p] = sin[i]
    return cos_dup, sin_signed


def _build_program():
    import concourse.bass as bass
    from concourse import bacc, mybir
    import concourse.tile as tile

    f32 = mybir.dt.float32
    f32r = mybir.dt.float32r
    bf16 = mybir.dt.bfloat16
    ADD = mybir.AluOpType.add
    MULT = mybir.AluOpType.mult
    EXP = mybir.ActivationFunctionType.Exp
    SWAP16 = [(j + 16) % 32 for j in range(32)]

    nc = bacc.Bacc("TRN2", target_bir_lowering=False, debug=False)
    xT = nc.dram_tensor("xT", [D, S], bf16, kind="ExternalInput").ap()
    wq = nc.dram_tensor("wq", [D, DC], bf16, kind="ExternalInput").ap()
    wk = nc.dram_tensor("wk", [D, DC], bf16, kind="ExternalInput").ap()
    wv = nc.dram_tensor("wv", [D, DC], bf16, kind="ExternalInput").ap()
    wo = nc.dram_tensor("wo", [DC, D], bf16, kind="ExternalInput").ap()
    cosd = nc.dram_tensor("cosd", [128, S], bf16, kind="ExternalInput").ap()
    sind = nc.dram_tensor("sind", [128, S], bf16, kind="ExternalInput").ap()
    sel = nc.dram_tensor("sel", [128, GQ * 128], f32r, kind="ExternalInput").ap()
    tri = nc.dram_tensor("tri", [KC, QT], f32, kind="ExternalInput").ap()
    vone = nc.dram_tensor("vone", [128, (S // KC) * HPC], f32r, kind="ExternalInput").ap()
    outT = nc.dram_tensor("outT", [D, S], bf16, kind="ExternalOutput").ap()

    with tile.TileContext(nc) as tc:
        with tc.tile_pool(name="const", bufs=1) as const:
            cos_sb = const.tile([128, S], bf16)
            sin_sb = const.tile([128, S], bf16)
            tri_sb = const.tile([KC, QT], f32)  # tri mask cols 0:128, zeros after
            sel_sb = const.tile([128, GQ * 128], f32r)
            wo_sb = const.tile([128, GQ, D], bf16)
            qT_sb = const.tile([128, GQ, S], f32r)
            kT_sb = const.tile([128, GQ, S], f32r)
            vaug_sb = const.tile([128, S // KC, HPC * (HD + 1)], f32r)
            oT_sb = const.tile([128, GQ, S], bf16)
            sums_sb = const.tile([128, S], f32)  # head h row at partition 32h
            recip_sb = const.tile([128, S], f32)
            recipr_sb = const.tile([128, S], f32r)
            nc.vector.memset(sums_sb, 1.0)
            nc.vector.memset(recip_sb, 1.0)

            with tc.tile_pool(name="px", bufs=1) as px, \
                 tc.tile_pool(name="rt", bufs=2) as rt, \
                 tc.tile_pool(name="prp", bufs=4) as prp, \
                 tc.tile_pool(name="evp", bufs=4) as evp:
                xT_sb = px.tile([128, NOC, S], bf16)
                wq_sb = px.tile([128, NOC, DC], bf16)
                wk_sb = px.tile([128, NOC, DC], bf16)
                wv_sb = px.tile([128, NOC, DC], bf16)
                # DMA spread: xT across 4 queues; weights need-ordered on
                # gpsimd; consts late on sync/scalar.
                xTr = xT.rearrange("(o p) n -> p o n", p=128)
                qeng = [nc.sync, nc.scalar]
                for oc in range(NOC):
                    qeng[oc % 2].dma_start(xT_sb[:, oc:oc + 1, :],
                                           xTr[:, oc:oc + 1, :])
                wkr = wk.rearrange("(o p) n -> p o n", p=128)
                wvr = wv.rearrange("(o p) n -> p o n", p=128)
                wqr = wq.rearrange("(o p) n -> p o n", p=128)
                for oc in range(NOC):  # per-chunk so chain kc=0 starts early
                    nc.gpsimd.dma_start(wk_sb[:, oc:oc + 1, :], wkr[:, oc:oc + 1, :])
                nc.gpsimd.dma_start(cos_sb[:, 0:QT], cosd[:, 0:QT])
                nc.gpsimd.dma_start(sin_sb[:, 0:QT], sind[:, 0:QT])
                for oc in range(NOC):
                    nc.sync.dma_start(wv_sb[:, oc:oc + 1, :], wvr[:, oc:oc + 1, :])
                for oc in range(NOC):
                    nc.scalar.dma_start(wq_sb[:, oc:oc + 1, :], wqr[:, oc:oc + 1, :])
                for qt in range(1, NQT):
                    nc.gpsimd.dma_start(cos_sb[:, qt * QT:(qt + 1) * QT],
                                        cosd[:, qt * QT:(qt + 1) * QT])
                    nc.gpsimd.dma_start(sin_sb[:, qt * QT:(qt + 1) * QT],
                                        sind[:, qt * QT:(qt + 1) * QT])
                # ones columns of v_aug (slot 64 of each head's 65-wide block)
                nc.gpsimd.dma_start(
                    vaug_sb[:, :, HD::(HD + 1)],
                    vone.rearrange("p (a b) -> p a b", a=S // KC))
                nc.gpsimd.dma_start(tri_sb, tri)
                nc.gpsimd.dma_start(sel_sb, sel)
                nc.scalar.dma_start(wo_sb, wo.rearrange("(o p) n -> p o n", p=128))

                def rope_into(ps, dst, q0):
                    shuf = rt.tile([128, QT], f32, tag="shuf", name="shuf")
                    nc.vector.stream_shuffle(shuf, ps, SWAP16)
                    m1 = rt.tile([128, QT], f32, tag="m1", name="m1")
                    nc.vector.tensor_tensor(m1, ps, cos_sb[:, q0:q0 + QT], MULT)
                    m2 = rt.tile([128, QT], f32, tag="m2", name="m2")
                    nc.vector.tensor_tensor(m2, shuf, sin_sb[:, q0:q0 + QT], MULT)
                    nc.vector.tensor_tensor(dst, m1, m2, ADD)

                def proj_chain_ops(w_sb, g, qt, dst_sb, psp, tag):
                    """8 accumulating matmuls + 1 rope bundle, as closures."""
                    q0 = qt * QT
                    cell = {}

                    def mk(kc):
                        def f():
                            if kc == 0:
                                cell["ps"] = psp.tile(
                                    [128, QT], f32, tag=tag, name=f"ps_{tag}")
                            nc.tensor.matmul(
                                cell["ps"], w_sb[:, kc, g * 128:(g + 1) * 128],
                                xT_sb[:, kc, q0:q0 + QT],
                                start=(kc == 0), stop=(kc == NOC - 1))
                        return f

                    ops = [mk(kc) for kc in range(NOC)]
                    ops.append(
                        lambda: rope_into(cell["ps"], dst_sb[:, g, q0:q0 + QT], q0))
                    return ops

                def vproj_ops(rc, psp):
                    cell = {}

                    def mk(kc):
                        def f():
                            if kc == 0:
                                cell["ps"] = psp.tile(
                                    [128, DC], f32, tag="v", name="ps_v")
                            nc.tensor.matmul(
                                cell["ps"], xT_sb[:, kc, rc * KC:(rc + 1) * KC],
                                wv_sb[:, kc, :],
                                start=(kc == 0), stop=(kc == NOC - 1))
                        return f

                    ops = [mk(kc) for kc in range(NOC)]

                    def evac():
                        src = cell["ps"].rearrange("p (h w) -> p h w", h=HPC)
                        dst = vaug_sb[:, rc, :].rearrange(
                            "p (h w) -> p h w", h=HPC)[:, :, 0:HD]
                        nc.vector.tensor_copy(out=dst, in_=src)

                    ops.append(evac)
                    return ops

                # ---------- pre-phase: K(g0), V, Q(g0, qt0) ----------
                with tc.tile_pool(name="ps_pre", bufs=2, space="PSUM") as psp:
                    for qt in range(NQT):
                        for op in proj_chain_ops(wk_sb, 0, qt, kT_sb, psp, "k"):
                            op()
                    for rc in range(S // KC):
                        for op in vproj_ops(rc, psp):
                            op()
                    for op in proj_chain_ops(wq_sb, 0, 0, qT_sb, psp, "q"):
                        op()

                # ---------- attention with interleaved fillers ----------
                with tc.tile_pool(name="ps_att", bufs=1, space="PSUM") as psa:
                    fill = deque()  # items: (deadline (g, qt) or None, op)

                    def emit_fillers(n):
                        for _ in range(min(n, len(fill))):
                            fill.popleft()[1]()

                    def flush_due(key):
                        # queue is enqueued in monotone deadline order
                        while fill and fill[0][0] is not None and fill[0][0] <= key:
                            fill.popleft()[1]()

                    def enq(dl, ops):
                        fill.extend((dl, op) for op in ops)

                    def attn_stream(g, qt):
                        """Joint stream over both head-halves: per pair p the
                        PE does 4 score MMs then 4 AV MMs of pair p-1, so each
                        EXP has a full round of slack -> PE-bound cadence."""
                        q0 = qt * QT
                        nkc = 4 * (qt + 1)
                        po = [psa.tile([HD + 1, QT], f32, tag=f"o{a}",
                                       name=f"ps_o{a}") for a in range(2)]

                        def emit_av(prs, p):
                            for a in range(2):
                                h = 2 * g + a
                                for ci in range(2):
                                    c = 2 * p + ci
                                    qlo = max(0, c * KC - q0)
                                    nc.tensor.matmul(
                                        po[a][:, qlo:QT],
                                        vaug_sb[:, c, h * (HD + 1):(h + 1) * (HD + 1)],
                                        prs[a][:, QT * ci + qlo:QT * (ci + 1)],
                                        start=(c == 0), stop=(c == nkc - 1),
                                        skip_group_check=True)

                        prev = None
                        for p in range(nkc // 2):
                            sps, prs = [], []
                            qlo0 = max(0, 2 * p * KC - q0)
                            for a in range(2):
                                sp = psa.tile([128, 2 * QT], f32,
                                              tag=f"spair{a}", name="spair")
                                sps.append(sp)
                                for ci in range(2):
                                    c = 2 * p + ci
                                    qlo = max(0, c * KC - q0)
                                    nc.tensor.matmul(
                                        sp[:, QT * ci + qlo:QT * (ci + 1)],
                                        kT_sb[a * HD:(a + 1) * HD, g,
                                              c * KC:(c + 1) * KC],
                                        qT_sb[a * HD:(a + 1) * HD, g,
                                              q0 + qlo:q0 + QT],
                                        start=True, stop=True,
                                        skip_group_check=True)
                            for a in range(2):
                                for ci in range(2):
                                    c = 2 * p + ci
                                    if c * KC >= q0:
                                        qlo = c * KC - q0
                                        nc.vector.tensor_tensor(
                                            sps[a][:, QT * ci + qlo:QT * ci + qlo + KC],
                                            sps[a][:, QT * ci + qlo:QT * ci + qlo + KC],
                                            tri_sb[:, 0:KC], ADD)
                            for a in range(2):
                                pr = prp.tile([128, 2 * QT], f32r, tag="probs",
                                              name="probs")
                                prs.append(pr)
                                nc.scalar.activation(pr[:, qlo0:],
                                                     sps[a][:, qlo0:], EXP)
                            if prev is not None:
                                emit_av(*prev)
                            emit_fillers(3)
                            prev = (prs, p)
                        emit_av(*prev)
                        # normalize both halves: sums row -> recip -> broadcast
                        # matmul into psb half -> oT half
                        psb = psa.tile([128, QT], f32, tag="pb", name="psb")
                        bb = rt.tile([128, QT], f32, tag="bb", name="bb")
                        for a in range(2):
                            row = 64 * g + 32 * a
                            nc.scalar.copy(
                                out=sums_sb[row:row + 1, q0:q0 + QT],
                                in_=po[a][HD:HD + 1, :])
                        nc.vector.reciprocal_approx_fast(
                            recip_sb[:, q0:q0 + QT], sums_sb[:, q0:q0 + QT])
                        nc.gpsimd.tensor_copy(
                            out=recipr_sb[:, q0:q0 + QT],
                            in_=recip_sb[:, q0:q0 + QT])
                        nc.tensor.matmul(
                            psb, sel_sb[:, g * 128:(g + 1) * 128],
                            recipr_sb[:, q0:q0 + QT], start=True, stop=True)
                        nc.scalar.copy(out=bb, in_=psb)
                        for a in range(2):
                            nc.vector.tensor_tensor(
                                oT_sb[64 * a:64 * (a + 1), g, q0:q0 + QT],
                                po[a][0:HD, :], bb[64 * a:64 * (a + 1), :], MULT)

                    def outproj_ops(qt, tail=False):
                        q0 = qt * QT
                        ops = []
                        for ec in range(NOC):
                            cell = {}
                            tag = ("pb" if (tail and ec % 2) else "pf")
                            evac_eng = nc.scalar if tail else nc.vector

                            def f1(ec=ec, cell=cell, tag=tag):
                                cell["ps"] = psa.tile([128, QT], f32, tag=tag,
                                                      name="ps_out")
                                nc.tensor.matmul(
                                    cell["ps"], wo_sb[:, 0, ec * 128:(ec + 1) * 128],
                                    oT_sb[:, 0, q0:q0 + QT],
                                    start=True, stop=False)

                            def f2(ec=ec, cell=cell, evac_eng=evac_eng):
                                nc.tensor.matmul(
                                    cell["ps"], wo_sb[:, 1, ec * 128:(ec + 1) * 128],
                                    oT_sb[:, 1, q0:q0 + QT],
                                    start=False, stop=True)
                                ob = evp.tile([128, QT], bf16, tag="ob", name="ob")
                                if evac_eng is nc.scalar:
                                    nc.scalar.copy(out=ob, in_=cell["ps"])
                                else:
                                    nc.vector.tensor_copy(out=ob, in_=cell["ps"])
                                nc.sync.dma_start(
                                    outT[ec * 128:(ec + 1) * 128, q0:q0 + QT], ob)

                            ops += [f1, f2]
                        if tail:
                            # software-pipeline: two chains (pf/pb) in flight
                            ordered = [ops[0], ops[2], ops[1]]
                            for ec in range(2, NOC):
                                ordered += [ops[2 * ec], ops[2 * ec - 1]]
                            ordered.append(ops[-1])
                            return ordered
                        return ops

                    # A0: attention g=0; fillers = Q(g0, qt1-3) then K/Q (g=1)
                    for qt in range(1, NQT):
                        enq((0, qt), proj_chain_ops(wq_sb, 0, qt, qT_sb, psa, "pf"))
                    for qt in range(NQT):
                        enq((1, qt), proj_chain_ops(wk_sb, 1, qt, kT_sb, psa, "pf"))
                        enq((1, qt), proj_chain_ops(wq_sb, 1, qt, qT_sb, psa, "pf"))
                    for qt in range(NQT):
                        flush_due((0, qt))
                        attn_stream(0, qt)

                    # A1: attention g=1; fillers += output projection
                    for qt in range(NQT):
                        flush_due((1, qt))
                        attn_stream(1, qt)
                        enq(None, outproj_ops(qt, tail=(qt == NQT - 1)))
                    emit_fillers(len(fill))

    nc.finalize()
    return nc


def kernel(x, wq, wk, wv, wo):
    import ml_dtypes
    from concourse import bass_utils

    bf16 = ml_dtypes.bfloat16

    if os.environ.get("BASS_TRACE"):
        _install_axon_ntff_hook()

    x = np.asarray(x, dtype=np.float32)
    wq = np.asarray(wq, dtype=np.float32)
    wk = np.asarray(wk, dtype=np.float32)
    wv = np.asarray(wv, dtype=np.float32)
    wo = np.asarray(wo, dtype=np.float32)

    # Host prep: weight slicing + rope column permutation + tables.
    perm_l = _rope_perm_local()
    perm = np.concatenate([h * HD + perm_l for h in range(NH)])  # [D]
    scale = 1.0 / np.sqrt(HD)
    wq_p = np.ascontiguousarray(wq[:, perm] * scale)
    wk_p = np.ascontiguousarray(wk[:, perm])
    cos_dup, sin_signed = _rope_tables()
    cos_dup = cos_dup.astype(bf16)
    sin_signed = sin_signed.astype(bf16)
    kl = np.arange(KC)[:, None]
    ql = np.arange(KC)[None, :]
    tri = np.zeros((KC, QT), np.float32)
    tri[:, :KC] = np.where(ql >= kl, 0.0, MASKVAL)
    # sel[64g+32a, 128g+64a : +64] = 1: broadcasts head (2g+a)'s recip row
    # (at partition 64g+32a) onto the 64 oT partitions of that head.
    sel = np.zeros((128, GQ * 128), np.float32)
    for g in range(GQ):
        for a in range(2):
            sel[64 * g + 32 * a, 128 * g + 64 * a:128 * g + 64 * a + 64] = 1.0

    xTs = [np.ascontiguousarray(x[b].T.astype(bf16)) for b in range(B)]
    vone = np.ones((128, (S // KC) * HPC), np.float32)

    in_maps = []
    for i in range(NCORES):
        b, g = divmod(i, HPC)
        cs = slice(g * DC, (g + 1) * DC)
        in_maps.append({
            "xT": xTs[b],
            "wq": np.ascontiguousarray(wq_p[:, cs].astype(bf16)),
            "wk": np.ascontiguousarray(wk_p[:, cs].astype(bf16)),
            "wv": np.ascontiguousarray(wv[:, cs].astype(bf16)),
            "wo": np.ascontiguousarray(wo[cs, :].astype(bf16)),
            "cosd": cos_dup,
            "sind": sin_signed,
            "tri": tri,
            "sel": sel,
            "vone": vone,
        })

    if "nc" not in _CACHE:
        _CACHE["nc"] = _build_program()
    nc = _CACHE["nc"]

    res = bass_utils.run_bass_kernel_spmd(nc, in_maps, core_ids=list(range(NCORES)))
    _CACHE["last_exec_time_ns"] = res.exec_time_ns
    _CACHE["last_res"] = res

    out = np.empty((B, S, D), dtype=np.float32)
    for b in range(B):
        acc = res.results[b * HPC]["outT"].astype(np.float32)
        for g in range(1, HPC):
            acc += res.results[b * HPC + g]["outT"].astype(np.float32)
        out[b] = acc.T
    return out


# revision 53
# speedup vs baseline: 1.0880x; 1.0880x over previous
"""Trainium2 Bass kernel for causal multi-head attention with interleaved RoPE.

Problem: B=2, S=2048, D=1024, 16 heads x 64 dims, causal, rope theta=1e4.

Sharding (8 cores): 2-way batch x 4-way head tensor-parallel.
  core i: batch b = i // 4, head group g = i % 4 (heads 4g..4g+3, dims 256).
  Each core computes q/k/v for its heads from x[b], runs causal flash
  attention, and produces a partial output projection outT = wo_g.T-slice
  contribution [D, S] in bf16.  Host sums the 4 partials per batch (f32) and
  transposes.

Performance structure (single gap-free PE stream to defeat the PE p-state
clock ramp):
  - pre-phase: K proj (g=0), V proj, Q proj (g=0), rope on DVE.
  - attention g=0, with the g=1 K/Q projection matmuls interleaved as PE
    fillers between attention rounds (they have no Scalar dependency, so
    they fill the EXP-bound gaps).
  - attention g=1, with the output-projection matmuls of completed q-tiles
    interleaved the same way.
  - Scores are computed transposed (S^T[k, q]) in chunk PAIRS sharing one
    2-bank PSUM tile so each Scalar EXP covers [128, 1024] (amortizes the
    ~185ns activation access bubble).  AV matmuls lag one pair behind the
    score matmuls (software pipeline), accumulating into a per-(g,qt,a)
    PSUM tile whose 65th row (ones column of v_aug) is the softmax
    normalizer.
  - Normalization: reciprocal_approx_fast of the sum rows, PE-matmul
    broadcast via a constant selector matrix, one DVE multiply per head
    half writing bf16 oT.
  - x / wq / wk / wv / wo travel as bf16 (halves DMA); q/k/scores/probs
    stay f32r.
"""

import os
import sys
from collections import deque

sys.path.insert(0, "/opt/trn_rl_repo")

import numpy as np

B = 2
S = 2048
D = 1024
NH = 16
HD = 64
THETA = 10000.0
NCORES = 8
HPC = 4  # heads per core
DC = HPC * HD  # 256 dims per core
GQ = 2  # 128-partition groups per core for q/k/o dims (DC/128)
QT = 512  # query tile (free dim)
NQT = S // QT
KC = 128  # key chunk (partition dim)
NOC = D // 128  # contraction chunks
MASKVAL = -60.0

_CACHE = {}


def _install_axon_ntff_hook():
    """Register antenv.axon_hooks so trace=True (BASS_TRACE=1) works."""
    import types

    if "antenv.axon_hooks" in sys.modules:
        return
    m = types.ModuleType("antenv.axon_hooks")
    _hook = [None]
    m.set_axon_ntff_profile_hook = lambda h: _hook.__setitem__(0, h)
    m.get_axon_ntff_profile_hook = lambda: _hook[0]
    sys.modules["antenv.axon_hooks"] = m
    try:
        import antenv

        antenv.axon_hooks = m
        from trn_agent_boot.trn_boot import _ntff_profile_via_ctypes

        hook = _ntff_profile_via_ctypes("/opt/axon/libaxon_pjrt.so")
        if hook is not None:
            m.set_axon_ntff_profile_hook(hook)
    except Exception:
        pass


def _rope_perm_local():
    """Permutation of one head's 64 dims: original interleaved pair (2i, 2i+1)
    -> t0 at quadrant*32 + (i%16), t1 at quadrant*32 + 16 + (i%16), with
    quadrant = i // 16.  Returns perm such that new[j] = old[perm[j]]."""
    perm = np.zeros(HD, dtype=np.int64)
    for i in range(HD // 2):
        qd, r = divmod(i, 16)
        perm[qd * 32 + r] = 2 * i
        perm[qd * 32 + 16 + r] = 2 * i + 1
    return perm


def _rope_tables():
    """cos_dup/sin_signed [128, S]: per-partition rope tables matching the
    de-interleaved layout (pattern repeats every 64 partitions)."""
    inv_freq = 1.0 / (THETA ** (np.arange(0, HD, 2, dtype=np.float64) / HD))  # [32]
    pos = np.arange(S, dtype=np.float64)
    ang = pos[None, :] * inv_freq[:, None]  # [32, S]
    cos = np.cos(ang)
    sin = np.sin(ang)
    cos_dup = np.zeros((128, S), dtype=np.float32)
    sin_signed = np.zeros((128, S), dtype=np.float32)
    for p in range(128):
        d = p % HD
        qd, r0 = divmod(d, 32)
        if r0 < 16:
            i = qd * 16 + r0
            cos_dup[p] = cos[i]
            sin_signed[p] = -sin[i]
        else:
            i = qd * 16 + (r0 - 16)
            cos_dup[p] = cos[i]
            sin_signed[p] = sin[i]
    return cos_dup, sin_signed


def _build_program():
    import concourse.bass as bass
    from concourse import bacc, mybir
    import concourse.tile as tile

    f32 = mybir.dt.float32
    f32r = mybir.dt.float32r
    bf16 = mybir.dt.bfloat16
    ADD = mybir.AluOpType.add
    MULT = mybir.AluOpType.mult
    EXP = mybir.ActivationFunctionType.Exp
    SWAP16 = [(j + 16) % 32 for j in range(32)]

    nc = bacc.Bacc("TRN2", target_bir_lowering=False, debug=False)
    xT = nc.dram_tensor("xT", [D, S], bf16, kind="ExternalInput").ap()
    wq = nc.dram_tensor("wq", [D, DC], bf16, kind="ExternalInput").ap()
    wk = nc.dram_tensor("wk", [D, DC], bf16, kind="ExternalInput").ap()
    wv = nc.dram_tensor("wv", [D, DC], bf16, kind="ExternalInput").ap()
    wo = nc.dram_tensor("wo", [DC, D], bf16, kind="ExternalInput").ap()
    cosd = nc.dram_tensor("cosd", [128, S], bf16, kind="ExternalInput").ap()
    sind = nc.dram_tensor("sind", [128, S], bf16, kind="ExternalInput").ap()
    sel = nc.dram_tensor("sel", [128, GQ * 128], f32r, kind="ExternalInput").ap()
    tri = nc.dram_tensor("tri", [KC, QT], f32, kind="ExternalInput").ap()
    vone = nc.dram_tensor("vone", [128, (S // KC) * HPC], f32r, kind="ExternalInput").ap()
    outT = nc.dram_tensor("outT", [D, S], bf16, kind="ExternalOutput").ap()

    with tile.TileContext(nc) as tc:
        with tc.tile_pool(name="const", bufs=1) as const:
            cos_sb = const.tile([128, S], bf16)
            sin_sb = const.tile([128, S], bf16)
            tri_sb = const.tile([KC, QT], f32)  # tri mask cols 0:128, zeros after
            sel_sb = const.tile([128, GQ * 128], f32r)
            wo_sb = const.tile([128, GQ, D], bf16)
            qT_sb = const.tile([128, GQ, S], f32r)
            kT_sb = const.tile([128, GQ, S], f32r)
            vaug_sb = const.tile([128, S // KC, HPC * (HD + 1)], f32r)
            oT_sb = const.tile([128, GQ, S], bf16)
            sums_sb = const.tile([128, S], f32)  # head h row at partition 32h
            recip_sb = const.tile([128, S], f32)
            recipr_sb = const.tile([128, S], f32r)
            nc.vector.memset(sums_sb, 1.0)
            nc.vector.memset(recip_sb, 1.0)

            with tc.tile_pool(name="px", bufs=1) as px, \
                 tc.tile_pool(name="rt", bufs=2) as rt, \
                 tc.tile_pool(name="prp", bufs=4) as prp, \
                 tc.tile_pool(name="evp", bufs=4) as evp:
                xT_sb = px.tile([128, NOC, S], bf16)
                wq_sb = px.tile([128, NOC, DC], bf16)
                wk_sb = px.tile([128, NOC, DC], bf16)
                wv_sb = px.tile([128, NOC, DC], bf16)
                # DMA spread: xT across 4 queues; weights need-ordered on
                # gpsimd; consts late on sync/scalar.
                xTr = xT.rearrange("(o p) n -> p o n", p=128)
                qeng = [nc.sync, nc.scalar]
                for oc in range(NOC):
                    qeng[oc % 2].dma_start(xT_sb[:, oc:oc + 1, :],
                                           xTr[:, oc:oc + 1, :])
                wkr = wk.rearrange("(o p) n -> p o n", p=128)
                wvr = wv.rearrange("(o p) n -> p o n", p=128)
                wqr = wq.rearrange("(o p) n -> p o n", p=128)
                for oc in range(NOC):  # per-chunk so chain kc=0 starts early
                    nc.gpsimd.dma_start(wk_sb[:, oc:oc + 1, :], wkr[:, oc:oc + 1, :])
                nc.gpsimd.dma_start(cos_sb[:, 0:QT], cosd[:, 0:QT])
                nc.gpsimd.dma_start(sin_sb[:, 0:QT], sind[:, 0:QT])
                for oc in range(NOC):
                    nc.sync.dma_start(wv_sb[:, oc:oc + 1, :], wvr[:, oc:oc + 1, :])
                for oc in range(NOC):
                    nc.scalar.dma_start(wq_sb[:, oc:oc + 1, :], wqr[:, oc:oc + 1, :])
                for qt in range(1, NQT):
                    nc.gpsimd.dma_start(cos_sb[:, qt * QT:(qt + 1) * QT],
                                        cosd[:, qt * QT:(qt + 1) * QT])
                    nc.gpsimd.dma_start(sin_sb[:, qt * QT:(qt + 1) * QT],
                                        sind[:, qt * QT:(qt + 1) * QT])
                # ones columns of v_aug (slot 64 of each head's 65-wide block)
                nc.gpsimd.dma_start(
                    vaug_sb[:, :, HD::(HD + 1)],
                    vone.rearrange("p (a b) -> p a b", a=S // KC))
                nc.gpsimd.dma_start(tri_sb, tri)
                nc.gpsimd.dma_start(sel_sb, sel)
                nc.scalar.dma_start(wo_sb, wo.rearrange("(o p) n -> p o n", p=128))

                def rope_into(ps, dst, q0):
                    shuf = rt.tile([128, QT], f32, tag="shuf", name="shuf")
                    nc.vector.stream_shuffle(shuf, ps, SWAP16)
                    m1 = rt.tile([128, QT], f32, tag="m1", name="m1")
                    nc.vector.tensor_tensor(m1, ps, cos_sb[:, q0:q0 + QT], MULT)
                    m2 = rt.tile([128, QT], f32, tag="m2", name="m2")
                    nc.vector.tensor_tensor(m2, shuf, sin_sb[:, q0:q0 + QT], MULT)
                    nc.vector.tensor_tensor(dst, m1, m2, ADD)

                def proj_chain_ops(w_sb, g, qt, dst_sb, psp, tag):
                    """8 accumulating matmuls + 1 rope bundle, as closures."""
                    q0 = qt * QT
                    cell = {}

                    def mk(kc):
                        def f():
                            if kc == 0:
                                cell["ps"] = psp.tile(
                                    [128, QT], f32, tag=tag, name=f"ps_{tag}")
                            nc.tensor.matmul(
                                cell["ps"], w_sb[:, kc, g * 128:(g + 1) * 128],
                                xT_sb[:, kc, q0:q0 + QT],
                                start=(kc == 0), stop=(kc == NOC - 1))
                        return f

                    ops = [mk(kc) for kc in range(NOC)]
                    ops.append(
                        lambda: rope_into(cell["ps"], dst_sb[:, g, q0:q0 + QT], q0))
                    return ops

                def vproj_ops(rc, psp):
                    cell = {}

                    def mk(kc):
                        def f():
                            if kc == 0:
                                cell["ps"] = psp.tile(
                                    [128, DC], f32, tag="v", name="ps_v")
                            nc.tensor.matmul(
                                cell["ps"], xT_sb[:, kc, rc * KC:(rc + 1) * KC],
                                wv_sb[:, kc, :],
                                start=(kc == 0), stop=(kc == NOC - 1))
                        return f

                    ops = [mk(kc) for kc in range(NOC)]

                    def evac():
                        src = cell["ps"].rearrange("p (h w) -> p h w", h=HPC)
                        dst = vaug_sb[:, rc, :].rearrange(
                            "p (h w) -> p h w", h=HPC)[:, :, 0:HD]
                        nc.vector.tensor_copy(out=dst, in_=src)

                    ops.append(evac)
                    return ops

                # ---------- pre-phase: K(g0), V, Q(g0, qt0) ----------
                with tc.tile_pool(name="ps_pre", bufs=2, space="PSUM") as psp:
                    for qt in range(NQT):
                        for op in proj_chain_ops(wk_sb, 0, qt, kT_sb, psp, "k"):
                            op()
                    for rc in range(S // KC):
                        for op in vproj_ops(rc, psp):
                            op()
                    for op in proj_chain_ops(wq_sb, 0, 0, qT_sb, psp, "q"):
                        op()

                # ---------- attention with interleaved fillers ----------
                with tc.tile_pool(name="ps_att", bufs=1, space="PSUM") as psa:
                    fill = deque()  # items: (deadline (g, qt) or None, op)

                    def emit_fillers(n):
                        for _ in range(min(n, len(fill))):
                            fill.popleft()[1]()

                    def flush_due(key):
                        # queue is enqueued in monotone deadline order
                        while fill and fill[0][0] is not None and fill[0][0] <= key:
                            fill.popleft()[1]()

                    def enq(dl, ops):
                        fill.extend((dl, op) for op in ops)

                    def attn_stream(g, qt):
                        """Joint stream over both head-halves: per pair p the
                        PE does 4 score MMs then 4 AV MMs of pair p-1, so each
                        EXP has a full round of slack -> PE-bound cadence."""
                        q0 = qt * QT
                        nkc = 4 * (qt + 1)
                        po = [psa.tile([HD + 1, QT], f32, tag=f"o{a}",
                                       name=f"ps_o{a}") for a in range(2)]

                        def emit_av(prs, p):
                            for a in range(2):
                                h = 2 * g + a
                                for ci in range(2):
                                    c = 2 * p + ci
                                    qlo = max(0, c * KC - q0)
                                    nc.tensor.matmul(
                                        po[a][:, qlo:QT],
                                        vaug_sb[:, c, h * (HD + 1):(h + 1) * (HD + 1)],
                                        prs[a][:, QT * ci + qlo:QT * (ci + 1)],
                                        start=(c == 0), stop=(c == nkc - 1),
                                        skip_group_check=True)

                        prev = None
                        for p in range(nkc // 2):
                            sps, prs = [], []
                            qlo0 = max(0, 2 * p * KC - q0)
                            for a in range(2):
                                sp = psa.tile([128, 2 * QT], f32,
                                              tag=f"spair{a}", name="spair")
                                sps.append(sp)
                                for ci in range(2):
                                    c = 2 * p + ci
                                    qlo = max(0, c * KC - q0)
                                    nc.tensor.matmul(
                                        sp[:, QT * ci + qlo:QT * (ci + 1)],
                                        kT_sb[a * HD:(a + 1) * HD, g,
                                              c * KC:(c + 1) * KC],
                                        qT_sb[a * HD:(a + 1) * HD, g,
                                              q0 + qlo:q0 + QT],
                                        start=True, stop=True,
                                        skip_group_check=True)
                            for a in range(2):
                                for ci in range(2):
                                    c = 2 * p + ci
                                    if c * KC >= q0:
                                        qlo = c * KC - q0
                                        nc.vector.tensor_tensor(
                                            sps[a][:, QT * ci + qlo:QT * ci + qlo + KC],
                                            sps[a][:, QT * ci + qlo:QT * ci + qlo + KC],
                                            tri_sb[:, 0:KC], ADD)
                            for a in range(2):
                                pr = prp.tile([128, 2 * QT], f32r, tag="probs",
                                              name="probs")
                                prs.append(pr)
                                nc.scalar.activation(pr[:, qlo0:],
                                                     sps[a][:, qlo0:], EXP)
                            if prev is not None:
                                emit_av(*prev)
                            emit_fillers(3)
                            prev = (prs, p)
                        emit_av(*prev)
                        # normalize both halves: sums row -> recip -> broadcast
                        # matmul into psb half -> oT half
                        psb = psa.tile([128, QT], f32, tag="pb", name="psb")
                        bb = rt.tile([128, QT], f32, tag="bb", name="bb")
                        for a in range(2):
                            row = 64 * g + 32 * a
                            nc.vector.tensor_copy(
                                out=sums_sb[row:row + 1, q0:q0 + QT],
                                in_=po[a][HD:HD + 1, :])
                        nc.vector.reciprocal_approx_fast(
                            recip_sb[:, q0:q0 + QT], sums_sb[:, q0:q0 + QT])
                        nc.gpsimd.tensor_copy(
                            out=recipr_sb[:, q0:q0 + QT],
                            in_=recip_sb[:, q0:q0 + QT])
                        nc.tensor.matmul(
                            psb, sel_sb[:, g * 128:(g + 1) * 128],
                            recipr_sb[:, q0:q0 + QT], start=True, stop=True)
                        nc.vector.tensor_copy(out=bb, in_=psb)
                        for a in range(2):
                            nc.vector.tensor_tensor(
                                oT_sb[64 * a:64 * (a + 1), g, q0:q0 + QT],
                                po[a][0:HD, :], bb[64 * a:64 * (a + 1), :], MULT)

                    def outproj_ops(qt, tail=False):
                        q0 = qt * QT
                        ops = []
                        for ec in range(NOC):
                            cell = {}
                            tag = ("pb" if (tail and ec % 2) else "pf")
                            evac_eng = nc.scalar if tail else nc.vector

                            def f1(ec=ec, cell=cell, tag=tag):
                                cell["ps"] = psa.tile([128, QT], f32, tag=tag,
                                                      name="ps_out")
                                nc.tensor.matmul(
                                    cell["ps"], wo_sb[:, 0, ec * 128:(ec + 1) * 128],
                                    oT_sb[:, 0, q0:q0 + QT],
                                    start=True, stop=False)

                            def f2(ec=ec, cell=cell, evac_eng=evac_eng):
                                nc.tensor.matmul(
                                    cell["ps"], wo_sb[:, 1, ec * 128:(ec + 1) * 128],
                                    oT_sb[:, 1, q0:q0 + QT],
                                    start=False, stop=True)
                                ob = evp.tile([128, QT], bf16, tag="ob", name="ob")
                                if evac_eng is nc.scalar:
                                    nc.scalar.copy(out=ob, in_=cell["ps"])
                                else:
                                    nc.vector.tensor_copy(out=ob, in_=cell["ps"])
                                nc.sync.dma_start(
                                    outT[ec * 128:(ec + 1) * 128, q0:q0 + QT], ob)

                            ops += [f1, f2]
                        if tail:
                            # software-pipeline: two chains (pf/pb) in flight
                            ordered = [ops[0], ops[2], ops[1]]
                            for ec in range(2, NOC):
                                ordered += [ops[2 * ec], ops[2 * ec - 1]]
                            ordered.append(ops[-1])
                            return ordered
                        return ops

                    # A0: attention g=0; fillers = Q(g0, qt1-3) then K/Q (g=1)
                    for qt in range(1, NQT):
                        enq((0, qt), proj_chain_ops(wq_sb, 0, qt, qT_sb, psa, "pf"))
                    for qt in range(NQT):
                        enq((1, qt), proj_chain_ops(wk_sb, 1, qt, kT_sb, psa, "pf"))
                        enq((1, qt), proj_chain_ops(wq_sb, 1, qt, qT_sb, psa, "pf"))
                    for qt in range(NQT):
                        flush_due((0, qt))
                        attn_stream(0, qt)

                    # A1: attention g=1; fillers += output projection
                    for qt in range(NQT):
                        flush_due((1, qt))
                        attn_stream(1, qt)
                        enq(None, outproj_ops(qt, tail=(qt == NQT - 1)))
                    emit_fillers(len(fill))

    nc.finalize()
    return nc


def kernel(x, wq, wk, wv, wo):
    import ml_dtypes
    from concourse import bass_utils

    bf16 = ml_dtypes.bfloat16

    if os.environ.get("BASS_TRACE"):
        _install_axon_ntff_hook()

    x = np.asarray(x, dtype=np.float32)
    wq = np.asarray(wq, dtype=np.float32)
    wk = np.asarray(wk, dtype=np.float32)
    wv = np.asarray(wv, dtype=np.float32)
    wo = np.asarray(wo, dtype=np.float32)

    # Host prep: weight slicing + rope column permutation + tables.
    perm_l = _rope_perm_local()
    perm = np.concatenate([h * HD + perm_l for h in range(NH)])  # [D]
    scale = 1.0 / np.sqrt(HD)
    wq_p = np.ascontiguousarray(wq[:, perm] * scale)
    wk_p = np.ascontiguousarray(wk[:, perm])
    cos_dup, sin_signed = _rope_tables()
    cos_dup = cos_dup.astype(bf16)
    sin_signed = sin_signed.astype(bf16)
    kl = np.arange(KC)[:, None]
    ql = np.arange(KC)[None, :]
    tri = np.zeros((KC, QT), np.float32)
    tri[:, :KC] = np.where(ql >= kl, 0.0, MASKVAL)
    # sel[64g+32a, 128g+64a : +64] = 1: broadcasts head (2g+a)'s recip row
    # (at partition 64g+32a) onto the 64 oT partitions of that head.
    sel = np.zeros((128, GQ * 128), np.float32)
    for g in range(GQ):
        for a in range(2):
            sel[64 * g + 32 * a, 128 * g + 64 * a:128 * g + 64 * a + 64] = 1.0

    xTs = [np.ascontiguousarray(x[b].T.astype(bf16)) for b in range(B)]
    vone = np.ones((128, (S // KC) * HPC), np.float32)

    in_maps = []
    for i in range(NCORES):
        b, g = divmod(i, HPC)
        cs = slice(g * DC, (g + 1) * DC)
        in_maps.append({
            "xT": xTs[b],
            "wq": np.ascontiguousarray(wq_p[:, cs].astype(bf16)),
            "wk": np.ascontiguousarray(wk_p[:, cs].astype(bf16)),
            "wv": np.ascontiguousarray(wv[:, cs].astype(bf16)),
            "wo": np.ascontiguousarray(wo[cs, :].astype(bf16)),
            "cosd": cos_dup,
            "sind": sin_signed,
            "tri": tri,
            "sel": sel,
            "vone": vone,
        })

    if "nc" not in _CACHE:
        _CACHE["nc"] = _build_program()
    nc = _CACHE["nc"]

    res = bass_utils.run_bass_kernel_spmd(nc, in_maps, core_ids=list(range(NCORES)))
    _CACHE["last_exec_time_ns"] = res.exec_time_ns
    _CACHE["last_res"] = res

    out = np.empty((B, S, D), dtype=np.float32)
    for b in range(B):
        acc = res.results[b * HPC]["outT"].astype(np.float32)
        for g in range(1, HPC):
            acc += res.results[b * HPC + g]["outT"].astype(np.float32)
        out[b] = acc.T
    return out


# revision 54
# speedup vs baseline: 1.2181x; 1.1196x over previous
"""Trainium2 Bass kernel for causal multi-head attention with interleaved RoPE.

Problem: B=2, S=2048, D=1024, 16 heads x 64 dims, causal, rope theta=1e4.

Sharding (8 cores): 2-way batch x 4-way head tensor-parallel.
  core i: batch b = i // 4, head group g = i % 4 (heads 4g..4g+3, dims 256).
  Each core computes q/k/v for its heads from x[b], runs causal flash
  attention, and produces a partial output projection outT = wo_g.T-slice
  contribution [D, S] in bf16.  Host sums the 4 partials per batch (f32) and
  transposes.

Performance structure (single gap-free PE stream to defeat the PE p-state
clock ramp):
  - pre-phase: K proj (g=0), V proj, Q proj (g=0), rope on DVE.
  - attention g=0, with the g=1 K/Q projection matmuls interleaved as PE
    fillers between attention rounds (they have no Scalar dependency, so
    they fill the EXP-bound gaps).
  - attention g=1, with the output-projection matmuls of completed q-tiles
    interleaved the same way.
  - Scores are computed transposed (S^T[k, q]) in chunk PAIRS sharing one
    2-bank PSUM tile so each Scalar EXP covers [128, 1024] (amortizes the
    ~185ns activation access bubble).  AV matmuls lag one pair behind the
    score matmuls (software pipeline), accumulating into a per-(g,qt,a)
    PSUM tile whose 65th row (ones column of v_aug) is the softmax
    normalizer.
  - Normalization: reciprocal_approx_fast of the sum rows, PE-matmul
    broadcast via a constant selector matrix, one DVE multiply per head
    half writing bf16 oT.
  - x / wq / wk / wv / wo travel as bf16 (halves DMA); q/k/scores/probs
    stay f32r.
"""

import os
import sys
from collections import deque

sys.path.insert(0, "/opt/trn_rl_repo")

import numpy as np

B = 2
S = 2048
D = 1024
NH = 16
HD = 64
THETA = 10000.0
NCORES = 8
HPC = 4  # heads per core
DC = HPC * HD  # 256 dims per core
GQ = 2  # 128-partition groups per core for q/k/o dims (DC/128)
QT = 512  # query tile (free dim)
NQT = S // QT
KC = 128  # key chunk (partition dim)
NOC = D // 128  # contraction chunks
MASKVAL = -60.0

_CACHE = {}


def _install_axon_ntff_hook():
    """Register antenv.axon_hooks so trace=True (BASS_TRACE=1) works."""
    import types

    if "antenv.axon_hooks" in sys.modules:
        return
    m = types.ModuleType("antenv.axon_hooks")
    _hook = [None]
    m.set_axon_ntff_profile_hook = lambda h: _hook.__setitem__(0, h)
    m.get_axon_ntff_profile_hook = lambda: _hook[0]
    sys.modules["antenv.axon_hooks"] = m
    try:
        import antenv

        antenv.axon_hooks = m
        from trn_agent_boot.trn_boot import _ntff_profile_via_ctypes

        hook = _ntff_profile_via_ctypes("/opt/axon/libaxon_pjrt.so")
        if hook is not None:
            m.set_axon_ntff_profile_hook(hook)
    except Exception:
        pass


def _rope_perm_local():
    """Permutation of one head's 64 dims: original interleaved pair (2i, 2i+1)
    -> t0 at quadrant*32 + (i%16), t1 at quadrant*32 + 16 + (i%16), with
    quadrant = i // 16.  Returns perm such that new[j] = old[perm[j]]."""
    perm = np.zeros(HD, dtype=np.int64)
    for i in range(HD // 2):
        qd, r = divmod(i, 16)
        perm[qd * 32 + r] = 2 * i
        perm[qd * 32 + 16 + r] = 2 * i + 1
    return perm


def _rope_tables():
    """cos_dup/sin_signed [128, S]: per-partition rope tables matching the
    de-interleaved layout (pattern repeats every 64 partitions)."""
    inv_freq = 1.0 / (THETA ** (np.arange(0, HD, 2, dtype=np.float64) / HD))  # [32]
    pos = np.arange(S, dtype=np.float64)
    ang = pos[None, :] * inv_freq[:, None]  # [32, S]
    cos = np.cos(ang)
    sin = np.sin(ang)
    cos_dup = np.zeros((128, S), dtype=np.float32)
    sin_signed = np.zeros((128, S), dtype=np.float32)
    for p in range(128):
        d = p % HD
        qd, r0 = divmod(d, 32)
        if r0 < 16:
            i = qd * 16 + r0
            cos_dup[p] = cos[i]
            sin_signed[p] = -sin[i]
        else:
            i = qd * 16 + (r0 - 16)
            cos_dup[p] = cos[i]
            sin_signed[p] = sin[i]
    return cos_dup, sin_signed


def _build_program():
    import concourse.bass as bass
    from concourse import bacc, mybir
    import concourse.tile as tile

    f32 = mybir.dt.float32
    f32r = mybir.dt.float32r
    bf16 = mybir.dt.bfloat16
    ADD = mybir.AluOpType.add
    MULT = mybir.AluOpType.mult
    EXP = mybir.ActivationFunctionType.Exp
    SWAP16 = [(j + 16) % 32 for j in range(32)]

    nc = bacc.Bacc("TRN2", target_bir_lowering=False, debug=False)
    xT = nc.dram_tensor("xT", [D, S], bf16, kind="ExternalInput").ap()
    wq = nc.dram_tensor("wq", [D, DC], bf16, kind="ExternalInput").ap()
    wk = nc.dram_tensor("wk", [D, DC], bf16, kind="ExternalInput").ap()
    wv = nc.dram_tensor("wv", [D, DC], bf16, kind="ExternalInput").ap()
    wo = nc.dram_tensor("wo", [DC, D], bf16, kind="ExternalInput").ap()
    cosd = nc.dram_tensor("cosd", [128, S], bf16, kind="ExternalInput").ap()
    sind = nc.dram_tensor("sind", [128, S], bf16, kind="ExternalInput").ap()
    sel = nc.dram_tensor("sel", [128, GQ * 128], f32r, kind="ExternalInput").ap()
    tri = nc.dram_tensor("tri", [KC, QT], f32, kind="ExternalInput").ap()
    vone = nc.dram_tensor("vone", [128, (S // KC) * HPC], f32r, kind="ExternalInput").ap()
    outT = nc.dram_tensor("outT", [D, S], bf16, kind="ExternalOutput").ap()

    with tile.TileContext(nc) as tc:
        with tc.tile_pool(name="const", bufs=1) as const:
            cos_sb = const.tile([128, S], bf16)
            sin_sb = const.tile([128, S], bf16)
            tri_sb = const.tile([KC, QT], f32)  # tri mask cols 0:128, zeros after
            sel_sb = const.tile([128, GQ * 128], f32r)
            wo_sb = const.tile([128, GQ, D], bf16)
            qT_sb = const.tile([128, GQ, S], f32r)
            kT_sb = const.tile([128, GQ, S], f32r)
            vaug_sb = const.tile([128, S // KC, HPC * (HD + 1)], f32r)
            oT_sb = const.tile([128, GQ, S], bf16)
            sums_sb = const.tile([128, S], f32)  # head h row at partition 32h
            recip_sb = const.tile([128, S], f32)
            recipr_sb = const.tile([128, S], f32r)
            nc.vector.memset(sums_sb, 1.0)
            nc.vector.memset(recip_sb, 1.0)

            with tc.tile_pool(name="px", bufs=1) as px, \
                 tc.tile_pool(name="rt", bufs=2) as rt, \
                 tc.tile_pool(name="prp", bufs=4) as prp, \
                 tc.tile_pool(name="evp", bufs=4) as evp:
                xT_sb = px.tile([128, NOC, S], bf16)
                wq_sb = px.tile([128, NOC, DC], bf16)
                wk_sb = px.tile([128, NOC, DC], bf16)
                wv_sb = px.tile([128, NOC, DC], bf16)
                # DMA spread: xT across 4 queues; weights need-ordered on
                # gpsimd; consts late on sync/scalar.
                xTr = xT.rearrange("(o p) n -> p o n", p=128)
                qeng = [nc.sync, nc.scalar]
                for oc in range(NOC):
                    qeng[oc % 2].dma_start(xT_sb[:, oc:oc + 1, :],
                                           xTr[:, oc:oc + 1, :])
                wkr = wk.rearrange("(o p) n -> p o n", p=128)
                wvr = wv.rearrange("(o p) n -> p o n", p=128)
                wqr = wq.rearrange("(o p) n -> p o n", p=128)
                for oc in range(NOC):  # per-chunk so chain kc=0 starts early
                    nc.gpsimd.dma_start(wk_sb[:, oc:oc + 1, :], wkr[:, oc:oc + 1, :])
                nc.gpsimd.dma_start(cos_sb[:, 0:QT], cosd[:, 0:QT])
                nc.gpsimd.dma_start(sin_sb[:, 0:QT], sind[:, 0:QT])
                for oc in range(NOC):
                    nc.sync.dma_start(wv_sb[:, oc:oc + 1, :], wvr[:, oc:oc + 1, :])
                for oc in range(NOC):
                    nc.scalar.dma_start(wq_sb[:, oc:oc + 1, :], wqr[:, oc:oc + 1, :])
                for qt in range(1, NQT):
                    nc.gpsimd.dma_start(cos_sb[:, qt * QT:(qt + 1) * QT],
                                        cosd[:, qt * QT:(qt + 1) * QT])
                    nc.gpsimd.dma_start(sin_sb[:, qt * QT:(qt + 1) * QT],
                                        sind[:, qt * QT:(qt + 1) * QT])
                # ones columns of v_aug (slot 64 of each head's 65-wide block)
                nc.gpsimd.dma_start(
                    vaug_sb[:, :, HD::(HD + 1)],
                    vone.rearrange("p (a b) -> p a b", a=S // KC))
                nc.gpsimd.dma_start(tri_sb, tri)
                nc.gpsimd.dma_start(sel_sb, sel)
                nc.scalar.dma_start(wo_sb, wo.rearrange("(o p) n -> p o n", p=128))

                def rope_into(ps, dst, q0):
                    shuf = rt.tile([128, QT], f32, tag="shuf", name="shuf")
                    nc.vector.stream_shuffle(shuf, ps, SWAP16)
                    m1 = rt.tile([128, QT], f32, tag="m1", name="m1")
                    nc.vector.tensor_tensor(m1, ps, cos_sb[:, q0:q0 + QT], MULT)
                    m2 = rt.tile([128, QT], f32, tag="m2", name="m2")
                    nc.vector.tensor_tensor(m2, shuf, sin_sb[:, q0:q0 + QT], MULT)
                    nc.vector.tensor_tensor(dst, m1, m2, ADD)

                def proj_chain_ops(w_sb, g, qt, dst_sb, psp, tag):
                    """8 accumulating matmuls + 1 rope bundle, as closures."""
                    q0 = qt * QT
                    cell = {}

                    def mk(kc):
                        def f():
                            if kc == 0:
                                cell["ps"] = psp.tile(
                                    [128, QT], f32, tag=tag, name=f"ps_{tag}")
                            nc.tensor.matmul(
                                cell["ps"], w_sb[:, kc, g * 128:(g + 1) * 128],
                                xT_sb[:, kc, q0:q0 + QT],
                                start=(kc == 0), stop=(kc == NOC - 1))
                        return f

                    ops = [mk(kc) for kc in range(NOC)]
                    ops.append(
                        lambda: rope_into(cell["ps"], dst_sb[:, g, q0:q0 + QT], q0))
                    return ops

                def vproj_ops(rc, psp):
                    cell = {}

                    def mk(kc):
                        def f():
                            if kc == 0:
                                cell["ps"] = psp.tile(
                                    [128, DC], f32, tag="v", name="ps_v")
                            nc.tensor.matmul(
                                cell["ps"], xT_sb[:, kc, rc * KC:(rc + 1) * KC],
                                wv_sb[:, kc, :],
                                start=(kc == 0), stop=(kc == NOC - 1))
                        return f

                    ops = [mk(kc) for kc in range(NOC)]

                    def evac():
                        src = cell["ps"].rearrange("p (h w) -> p h w", h=HPC)
                        dst = vaug_sb[:, rc, :].rearrange(
                            "p (h w) -> p h w", h=HPC)[:, :, 0:HD]
                        nc.vector.tensor_copy(out=dst, in_=src)

                    ops.append(evac)
                    return ops

                # ---------- pre-phase: K(g0), V, Q(g0, qt0) ----------
                with tc.tile_pool(name="ps_pre", bufs=2, space="PSUM") as psp:
                    for qt in range(NQT):
                        for op in proj_chain_ops(wk_sb, 0, qt, kT_sb, psp, "k"):
                            op()
                    for rc in range(S // KC):
                        for op in vproj_ops(rc, psp):
                            op()
                    for op in proj_chain_ops(wq_sb, 0, 0, qT_sb, psp, "q"):
                        op()

                # ---------- attention with interleaved fillers ----------
                with tc.tile_pool(name="ps_att", bufs=1, space="PSUM") as psa:
                    fill = deque()  # items: (deadline (g, qt) or None, op)

                    def emit_fillers(n):
                        for _ in range(min(n, len(fill))):
                            fill.popleft()[1]()

                    def flush_due(key):
                        # queue is enqueued in monotone deadline order
                        while fill and fill[0][0] is not None and fill[0][0] <= key:
                            fill.popleft()[1]()

                    def enq(dl, ops):
                        fill.extend((dl, op) for op in ops)

                    def attn_stream(g, qt):
                        """Joint stream over both head-halves: per pair p the
                        PE does 4 score MMs then 4 AV MMs of pair p-1, so each
                        EXP has a full round of slack -> PE-bound cadence."""
                        q0 = qt * QT
                        nkc = 4 * (qt + 1)
                        po = [psa.tile([HD + 1, QT], f32, tag=f"o{a}",
                                       name=f"ps_o{a}") for a in range(2)]

                        def emit_av(prs, p):
                            for a in range(2):
                                h = 2 * g + a
                                for ci in range(2):
                                    c = 2 * p + ci
                                    qlo = max(0, c * KC - q0)
                                    nc.tensor.matmul(
                                        po[a][:, qlo:QT],
                                        vaug_sb[:, c, h * (HD + 1):(h + 1) * (HD + 1)],
                                        prs[a][:, QT * ci + qlo:QT * (ci + 1)],
                                        start=(c == 0), stop=(c == nkc - 1),
                                        skip_group_check=True)

                        prev = None
                        for p in range(nkc // 2):
                            sps, prs = [], []
                            qlo0 = max(0, 2 * p * KC - q0)
                            for a in range(2):
                                sp = psa.tile([128, 2 * QT], f32,
                                              tag=f"spair{a}", name="spair")
                                sps.append(sp)
                                for ci in range(2):
                                    c = 2 * p + ci
                                    qlo = max(0, c * KC - q0)
                                    nc.tensor.matmul(
                                        sp[:, QT * ci + qlo:QT * (ci + 1)],
                                        kT_sb[a * HD:(a + 1) * HD, g,
                                              c * KC:(c + 1) * KC],
                                        qT_sb[a * HD:(a + 1) * HD, g,
                                              q0 + qlo:q0 + QT],
                                        start=True, stop=True,
                                        skip_group_check=True)
                            for a in range(2):
                                for ci in range(2):
                                    c = 2 * p + ci
                                    if c * KC >= q0:
                                        qlo = c * KC - q0
                                        nc.vector.tensor_tensor(
                                            sps[a][:, QT * ci + qlo:QT * ci + qlo + KC],
                                            sps[a][:, QT * ci + qlo:QT * ci + qlo + KC],
                                            tri_sb[:, 0:KC], ADD)
                            for a in range(2):
                                pr = prp.tile([128, 2 * QT], f32r, tag="probs",
                                              name="probs")
                                prs.append(pr)
                                nc.scalar.activation(pr[:, qlo0:],
                                                     sps[a][:, qlo0:], EXP)
                            if prev is not None:
                                emit_av(*prev)
                            emit_fillers(3)
                            prev = (prs, p)
                        emit_av(*prev)
                        # normalize both halves: sums row -> recip -> broadcast
                        # matmul into psb half -> oT half
                        psb = psa.tile([128, QT], f32, tag="pb", name="psb")
                        bb = rt.tile([128, QT], f32, tag="bb", name="bb")
                        for a in range(2):
                            row = 64 * g + 32 * a
                            nc.vector.tensor_copy(
                                out=sums_sb[row:row + 1, q0:q0 + QT],
                                in_=po[a][HD:HD + 1, :])
                        nc.vector.reciprocal_approx_fast(
                            recip_sb[:, q0:q0 + QT], sums_sb[:, q0:q0 + QT])
                        nc.vector.tensor_copy(
                            out=recipr_sb[:, q0:q0 + QT],
                            in_=recip_sb[:, q0:q0 + QT])
                        nc.tensor.matmul(
                            psb, sel_sb[:, g * 128:(g + 1) * 128],
                            recipr_sb[:, q0:q0 + QT], start=True, stop=True)
                        nc.vector.tensor_copy(out=bb, in_=psb)
                        for a in range(2):
                            nc.vector.tensor_tensor(
                                oT_sb[64 * a:64 * (a + 1), g, q0:q0 + QT],
                                po[a][0:HD, :], bb[64 * a:64 * (a + 1), :], MULT)

                    def outproj_ops(qt, tail=False):
                        q0 = qt * QT
                        ops = []
                        for ec in range(NOC):
                            cell = {}
                            tag = ("pb" if (tail and ec % 2) else "pf")
                            evac_eng = nc.scalar if tail else nc.vector

                            def f1(ec=ec, cell=cell, tag=tag):
                                cell["ps"] = psa.tile([128, QT], f32, tag=tag,
                                                      name="ps_out")
                                nc.tensor.matmul(
                                    cell["ps"], wo_sb[:, 0, ec * 128:(ec + 1) * 128],
                                    oT_sb[:, 0, q0:q0 + QT],
                                    start=True, stop=False)

                            def f2(ec=ec, cell=cell, evac_eng=evac_eng):
                                nc.tensor.matmul(
                                    cell["ps"], wo_sb[:, 1, ec * 128:(ec + 1) * 128],
                                    oT_sb[:, 1, q0:q0 + QT],
                                    start=False, stop=True)
                                ob = evp.tile([128, QT], bf16, tag="ob", name="ob")
                                if evac_eng is nc.scalar:
                                    nc.scalar.copy(out=ob, in_=cell["ps"])
                                else:
                                    nc.vector.tensor_copy(out=ob, in_=cell["ps"])
                                nc.sync.dma_start(
                                    outT[ec * 128:(ec + 1) * 128, q0:q0 + QT], ob)

                            ops += [f1, f2]
                        if tail:
                            # software-pipeline: two chains (pf/pb) in flight
                            ordered = [ops[0], ops[2], ops[1]]
                            for ec in range(2, NOC):
                                ordered += [ops[2 * ec], ops[2 * ec - 1]]
                            ordered.append(ops[-1])
                            return ordered
                        return ops

                    # A0: attention g=0; fillers = Q(g0, qt1-3) then K/Q (g=1)
                    for qt in range(1, NQT):
                        enq((0, qt), proj_chain_ops(wq_sb, 0, qt, qT_sb, psa, "pf"))
                    for qt in range(NQT):
                        enq((1, qt), proj_chain_ops(wk_sb, 1, qt, kT_sb, psa, "pf"))
                        enq((1, qt), proj_chain_ops(wq_sb, 1, qt, qT_sb, psa, "pf"))
                    for qt in range(NQT):
                        flush_due((0, qt))
                        attn_stream(0, qt)

                    # A1: attention g=1; fillers += output projection
                    for qt in range(NQT):
                        flush_due((1, qt))
                        attn_stream(1, qt)
                        enq(None, outproj_ops(qt, tail=(qt == NQT - 1)))
                    emit_fillers(len(fill))

    nc.finalize()
    return nc


def kernel(x, wq, wk, wv, wo):
    import ml_dtypes
    from concourse import bass_utils

    bf16 = ml_dtypes.bfloat16

    if os.environ.get("BASS_TRACE"):
        _install_axon_ntff_hook()

    x = np.asarray(x, dtype=np.float32)
    wq = np.asarray(wq, dtype=np.float32)
    wk = np.asarray(wk, dtype=np.float32)
    wv = np.asarray(wv, dtype=np.float32)
    wo = np.asarray(wo, dtype=np.float32)

    # Host prep: weight slicing + rope column permutation + tables.
    perm_l = _rope_perm_local()
    perm = np.concatenate([h * HD + perm_l for h in range(NH)])  # [D]
    scale = 1.0 / np.sqrt(HD)
    wq_p = np.ascontiguousarray(wq[:, perm] * scale)
    wk_p = np.ascontiguousarray(wk[:, perm])
    cos_dup, sin_signed = _rope_tables()
    cos_dup = cos_dup.astype(bf16)
    sin_signed = sin_signed.astype(bf16)
    kl = np.arange(KC)[:, None]
    ql = np.arange(KC)[None, :]
    tri = np.zeros((KC, QT), np.float32)
    tri[:, :KC] = np.where(ql >= kl, 0.0, MASKVAL)
    # sel[64g+32a, 128g+64a : +64] = 1: broadcasts head (2g+a)'s recip row
    # (at partition 64g+32a) onto the 64 oT partitions of that head.
    sel = np.zeros((128, GQ * 128), np.float32)
    for g in range(GQ):
        for a in range(2):
            sel[64 * g + 32 * a, 128 * g + 64 * a:128 * g + 64 * a + 64] = 1.0

    xTs = [np.ascontiguousarray(x[b].T.astype(bf16)) for b in range(B)]
    vone = np.ones((128, (S // KC) * HPC), np.float32)

    in_maps = []
    for i in range(NCORES):
        b, g = divmod(i, HPC)
        cs = slice(g * DC, (g + 1) * DC)
        in_maps.append({
            "xT": xTs[b],
            "wq": np.ascontiguousarray(wq_p[:, cs].astype(bf16)),
            "wk": np.ascontiguousarray(wk_p[:, cs].astype(bf16)),
            "wv": np.ascontiguousarray(wv[:, cs].astype(bf16)),
            "wo": np.ascontiguousarray(wo[cs, :].astype(bf16)),
            "cosd": cos_dup,
            "sind": sin_signed,
            "tri": tri,
            "sel": sel,
            "vone": vone,
        })

    if "nc" not in _CACHE:
        _CACHE["nc"] = _build_program()
    nc = _CACHE["nc"]

    res = bass_utils.run_bass_kernel_spmd(nc, in_maps, core_ids=list(range(NCORES)))
    _CACHE["last_exec_time_ns"] = res.exec_time_ns
    _CACHE["last_res"] = res

    out = np.empty((B, S, D), dtype=np.float32)
    for b in range(B):
        acc = res.results[b * HPC]["outT"].astype(np.float32)
        for g in range(1, HPC):
            acc += res.results[b * HPC + g]["outT"].astype(np.float32)
        out[b] = acc.T
    return out
